# revision 53
# baseline (speedup 1.0000x reference)
"""Trainium2 Bass kernel for nn_DeltaDNNGLM (truncated-BPTT delta-rule GLM forward).

Reference computation (per batch element b, sequential over t = 0..S-1):
    z_t   = concat(W_t, x[b, t, 1:])                # (2 + 62,) = (64,)
    h1    = relu(z_t @ W1 + b1)                     # (512,)
    h2    = relu(h1 @ W2 + b2)                      # (512,)
    dW_t  = 0.001 * (h2 @ W3 + b3)                  # (2,)
    W_t+1 = W_t + dW_t
Outputs:
    outputs[b, t, :]   = W_{t+1}
    sigmoid_output     = sigmoid(outputs[..,0]*x[..,0] + outputs[..,1])
    dW_last            = dW_{S-1}

Strategy: pure data parallel over batch (1024 = 8 cores x 128).  Per core the
activations are kept feature-major: (hidden-on-partitions, batch-on-free), so
every matmul uses the weights as the stationary operand and streams the batch
(N=128).  The x-dependent part of layer 1 is batched 2 timesteps per matmul
(N=256) with b1 folded in via a ones feature-plane; b2/b3 biases ride rank-1
matmuls / the state-update's per-partition scalar.  The W-state recurrence is
kept in SBUF fp32 (plus a bf16 copy feeding layer 1); matmuls run in bf16 with
fp32 PSUM accumulation; dW_last is recomputed fully in fp32 for the final step
(catastrophic cancellation).  Critical chain per step (~3.9 us in the
concourse cost-model timeline; all other work hides under it):
  z-update(DVE) -> L1w(PE,4MM) -> relu1(DVE) -> L2(PE,16MM) -> relu2(DVE)
  -> L3(PE,4MM) -> z-update
"""

import os

import numpy as np
import ml_dtypes

import concourse.bass as bass
import concourse.bacc as bacc
import concourse.mybir as mybir
import concourse.tile as tile
from concourse.bass import ts
from concourse.bass_utils import run_bass_kernel_spmd

B, S_FULL, F, H, OUT = 1024, 1000, 63, 512, 2
NCORES = 8
BL = B // NCORES          # 128 batch per core
TCH = 100                 # chunk (truncation) length
NJ = H // 128             # 4 hidden blocks
TB = 2                    # timesteps batched per layer-1-x matmul

f32 = mybir.dt.float32
bf16 = mybir.dt.bfloat16
BF = ml_dtypes.bfloat16
AF = mybir.ActivationFunctionType
ALU = mybir.AluOpType


def _emit_fp32_last_step(nc, const, hpool, pB, pW, sigpool, w_prev,
                         w1xf_d, w1wf_d, w2f_d, w3f_d, b3f_d, xlast_d,
                         b1c, b2c, dw_out):
    """Recompute dW at t=S-1 entirely in fp32 (only dw_out depends on it)."""
    w1xf = const.tile([F - 1, H], f32)
    nc.sync.dma_start(out=w1xf[:], in_=w1xf_d[:])
    w1wf = const.tile([OUT, H], f32)
    nc.sync.dma_start(out=w1wf[:], in_=w1wf_d[:])
    w2f = const.tile([128, NJ, H], f32)
    nc.sync.dma_start(out=w2f[:], in_=w2f_d[:].rearrange("kb kp j -> kp kb j"))
    w3f = const.tile([128, NJ, OUT], f32)
    nc.sync.dma_start(out=w3f[:], in_=w3f_d[:].rearrange("kb kp o -> kp kb o"))
    b3f = const.tile([1, OUT], f32)
    nc.sync.dma_start(out=b3f[:], in_=b3f_d[:])
    xlast = const.tile([F - 1, BL], f32)
    nc.sync.dma_start(out=xlast[:], in_=xlast_d[:])
    onesf = const.tile([1, BL], f32)
    nc.vector.memset(onesf[:], 1.0)

    paf = pB.tile([128, NJ, BL], f32, tag="pb")
    for j in range(NJ):
        nc.tensor.matmul(paf[:, j], w1xf[:, ts(j, 128)], xlast[:],
                         start=(j == 0), stop=False)
        nc.tensor.matmul(paf[:, j], w1wf[:, ts(j, 128)], w_prev,
                         start=False, stop=True)
    h1f = hpool.tile([128, H], f32, tag="h1f")
    for j in range(NJ):
        nc.scalar.activation(out=h1f[:, ts(j, 128)], in_=paf[:, j],
                             func=AF.Relu, bias=b1c[:, j:j + 1], scale=1.0)
    pbf = pB.tile([128, NJ, BL], f32, tag="pb")
    for kb in range(NJ):
        for j in range(NJ):
            nc.tensor.matmul(pbf[:, j], w2f[:, kb, ts(j, 128)],
                             h1f[:, ts(kb, 128)],
                             start=(kb == 0 and j == 0), stop=(kb == NJ - 1))
    h2f = hpool.tile([128, H], f32, tag="h2f")
    for j in range(NJ):
        nc.scalar.activation(out=h2f[:, ts(j, 128)], in_=pbf[:, j],
                             func=AF.Relu, bias=b2c[:, j:j + 1], scale=1.0)
    pwf = pW.tile([OUT, BL], f32, tag="pw")
    for kb in range(NJ):
        nc.tensor.matmul(pwf[:], w3f[:, kb], h2f[:, ts(kb, 128)],
                         start=(kb == 0), stop=False)
    nc.tensor.matmul(pwf[:], b3f[:], onesf[:], start=False, stop=True)
    dwsb = sigpool.tile([OUT, BL], f32)
    nc.vector.tensor_copy(out=dwsb[:], in_=pwf[:])
    nc.sync.dma_start(out=dw_out[:], in_=dwsb[:])


def build(S=S_FULL, debug=False):
    nch = S // TCH
    nc = bacc.Bacc(None, target_bir_lowering=False)

    # ---- DRAM I/O ----
    # feature rows 0..61 = x[:, t, 1:63]; row 62 = ones (carries b1 via lhsT)
    x_mlp = nc.declare_dram_parameter("x_mlp", [nch, F, TCH, BL], bf16, False)
    x0 = nc.declare_dram_parameter("x0", [nch, TCH, BL], f32, False)
    w0t = nc.declare_dram_parameter("w0t", [OUT, BL], f32, False)
    w1x_d = nc.declare_dram_parameter("w1x", [F, H], bf16, False)  # [W1x; b1]
    w1w_d = nc.declare_dram_parameter("w1w", [OUT, H], bf16, False)
    w2_d = nc.declare_dram_parameter("w2", [NJ, 128, H], bf16, False)
    w3_d = nc.declare_dram_parameter("w3", [NJ, 128, OUT], bf16, False)  # pre-scaled by 1e-3
    b1c_d = nc.declare_dram_parameter("b1col", [128, NJ], f32, False)
    b3c_d = nc.declare_dram_parameter("b3col", [OUT, 1], f32, False)   # pre-scaled
    b2c_d = nc.declare_dram_parameter("b2col", [128, NJ], f32, False)
    b1r_d = nc.declare_dram_parameter("b1row", [1, H], bf16, False)
    b2r_d = nc.declare_dram_parameter("b2row", [1, H], bf16, False)
    b3_d = nc.declare_dram_parameter("b3row", [1, OUT], bf16, False)     # pre-scaled by 1e-3
    # fp32 weight copies + last-step x column for the fp32 shadow of step S-1
    # (dW_last suffers catastrophic cancellation at bf16 precision)
    w1xf_d = nc.declare_dram_parameter("w1x_f", [F - 1, H], f32, False)
    w1wf_d = nc.declare_dram_parameter("w1w_f", [OUT, H], f32, False)
    w2f_d = nc.declare_dram_parameter("w2_f", [NJ, 128, H], f32, False)
    w3f_d = nc.declare_dram_parameter("w3_f", [NJ, 128, OUT], f32, False)
    b3f_d = nc.declare_dram_parameter("b3_f", [1, OUT], f32, False)
    xlast_d = nc.declare_dram_parameter("x_last", [F - 1, BL], f32, False)
    ws_out = nc.declare_dram_parameter("ws_out", [nch, OUT, TCH, BL], f32, True)
    sig_out = nc.declare_dram_parameter("sig_out", [nch, TCH, BL], f32, True)
    dw_out = nc.declare_dram_parameter("dw_out", [OUT, BL], f32, True)
    if debug:
        dbg_pa = nc.declare_dram_parameter("dbg_pa", [128, NJ, BL], f32, True)
        dbg_h1 = nc.declare_dram_parameter("dbg_h1", [128, NJ, BL], f32, True)
        dbg_pb = nc.declare_dram_parameter("dbg_pb", [128, NJ, BL], f32, True)
        dbg_h2 = nc.declare_dram_parameter("dbg_h2", [128, NJ, BL], f32, True)

    with tile.TileContext(nc) as tc:
        with (
            tc.tile_pool(name="const", bufs=1) as const,
            tc.tile_pool(name="xpool", bufs=2) as xpool,
            tc.tile_pool(name="wspool", bufs=2) as wspool,
            tc.tile_pool(name="hpool", bufs=2) as hpool,
            tc.tile_pool(name="sigpool", bufs=2) as sigpool,
            tc.tile_pool(name="pA", bufs=2, space="PSUM") as pA,
            tc.tile_pool(name="pB", bufs=2, space="PSUM") as pB,
            tc.tile_pool(name="pW", bufs=2, space="PSUM") as pW,
        ):
            # ---- load constants ----
            w1x = const.tile([F, H], bf16)
            nc.sync.dma_start(out=w1x[:], in_=w1x_d[:])
            w1w = const.tile([OUT, H], bf16)
            nc.sync.dma_start(out=w1w[:], in_=w1w_d[:])
            b2r = const.tile([1, H], bf16)
            nc.sync.dma_start(out=b2r[:], in_=b2r_d[:])
            w2 = const.tile([128, NJ, H], bf16)
            nc.sync.dma_start(out=w2[:], in_=w2_d[:].rearrange("kb kp j -> kp kb j"))
            w3 = const.tile([128, NJ, OUT], bf16)
            nc.sync.dma_start(out=w3[:], in_=w3_d[:].rearrange("kb kp o -> kp kb o"))
            b1c = const.tile([128, NJ], f32)
            nc.sync.dma_start(out=b1c[:], in_=b1c_d[:])
            b2c = const.tile([128, NJ], f32)
            nc.sync.dma_start(out=b2c[:], in_=b2c_d[:])
            b3r = const.tile([1, OUT], bf16)
            nc.sync.dma_start(out=b3r[:], in_=b3_d[:])
            b3c = const.tile([OUT, 1], f32)
            nc.sync.dma_start(out=b3c[:], in_=b3c_d[:])
            w0sb = const.tile([OUT, BL], f32)
            nc.sync.dma_start(out=w0sb[:], in_=w0t[:])
            ones2 = const.tile([1, TB * BL], bf16)
            nc.vector.memset(ones2[:], 1.0)

            z = const.tile([OUT, BL], bf16)   # bf16 copy of W(t) for layer-1 matmul
            nc.vector.tensor_copy(out=z[:], in_=w0sb[:])
            w_cur = w0sb[:]  # AP of W(t) in SBUF fp32
            pending_ws = None

            for ch in range(nch):
                x_t = xpool.tile([F, TCH, BL], bf16)
                nc.sync.dma_start(out=x_t[:], in_=x_mlp[ch])
                ws_t = wspool.tile([OUT, TCH, BL], f32)

                for tb in range(TCH // TB):
                    pa = pA.tile([128, NJ, TB, BL], f32)
                    # layer-1 x-part: one matmul per hidden block, TB steps wide.
                    # start=True clears the whole PSUM *bank*, so only the
                    # first matmul touching each bank may set it (2 j per bank).
                    for j in range(NJ):
                        nc.tensor.matmul(
                            pa[:, j], w1x[:, ts(j, 128)],
                            x_t[:, TB * tb:TB * (tb + 1), :],
                            start=(j % 2 == 0), stop=False,
                        )
                    for tloc in range(TB):
                        tl = TB * tb + tloc
                        # layer-1 W-state part (K=2, bf16 via z state)
                        for j in range(NJ):
                            nc.tensor.matmul(
                                pa[:, j, tloc], w1w[:, ts(j, 128)], z[:],
                                start=False, stop=(tloc == TB - 1),
                            )
                        # b2 bias MMs early: they clear pb's bank and run
                        # in PE's idle window during relu1
                        pb = pB.tile([128, NJ, BL], f32)
                        for j in range(NJ):
                            nc.tensor.matmul(
                                pb[:, j], b2r[:, ts(j, 128)], ones2[:, 0:BL],
                                start=(j == 0), stop=False,
                            )
                        # relu1 -> h1 (bf16); bias already in PSUM. Single ACT op.
                        h1 = hpool.tile([128, NJ, BL], bf16)
                        nc.vector.tensor_scalar(
                            out=h1[:], in0=pa[:, :, tloc, :],
                            scalar1=0.0, scalar2=None, op0=ALU.max,
                        )
                        # layer 2
                        for kb in range(NJ):
                            for j in range(NJ):
                                nc.tensor.matmul(
                                    pb[:, j], w2[:, kb, ts(j, 128)],
                                    h1[:, kb, :],
                                    start=False, stop=(kb == NJ - 1),
                                )
                        if pending_ws is not None:
                            _pw, _wc, _out = pending_ws
                            nc.vector.scalar_tensor_tensor(
                                out=_out, in0=_pw[:], scalar=b3c[:, 0:1],
                                in1=_wc, op0=ALU.add, op1=ALU.add,
                            )
                            pending_ws = None
                        # relu2 -> h2 (bf16), single DVE op
                        h2 = hpool.tile([128, NJ, BL], bf16)
                        nc.vector.tensor_scalar(
                            out=h2[:], in0=pb[:],
                            scalar1=0.0, scalar2=None, op0=ALU.max,
                        )
                        if debug and ch == 0 and tl == 0:
                            dpa = sigpool.tile([128, NJ, BL], f32, tag="dbgpa")
                            nc.vector.tensor_copy(out=dpa[:], in_=pa[:, :, tloc, :])
                            nc.sync.dma_start(out=dbg_pa[:], in_=dpa[:])
                            dh1 = sigpool.tile([128, NJ, BL], f32, tag="dbgh1")
                            nc.vector.tensor_copy(out=dh1[:], in_=h1[:])
                            nc.sync.dma_start(out=dbg_h1[:], in_=dh1[:])
                            dpb = sigpool.tile([128, NJ, BL], f32, tag="dbgpb")
                            nc.vector.tensor_copy(out=dpb[:], in_=pb[:])
                            nc.sync.dma_start(out=dbg_pb[:], in_=dpb[:])
                            dh2 = sigpool.tile([128, NJ, BL], f32, tag="dbgh2")
                            nc.vector.tensor_copy(out=dh2[:], in_=h2[:])
                            nc.sync.dma_start(out=dbg_h2[:], in_=dh2[:])
                        # layer 3 (+ scaled bias via ones row) -> dW in PSUM
                        pw = pW.tile([OUT, BL], f32)
                        for kb in range(NJ):
                            nc.tensor.matmul(
                                pw[:], w3[:, kb], h2[:, kb, :],
                                start=(kb == 0), stop=(kb == NJ - 1),
                            )
                        # state update: z (bf16, critical path) then ws (fp32)
                        nc.vector.scalar_tensor_tensor(
                            out=z[:], in0=pw[:], scalar=b3c[:, 0:1],
                            in1=w_cur, op0=ALU.add, op1=ALU.add,
                        )
                        pending_ws = (pw, w_cur, ws_t[:, tl, :])
                        w_prev = w_cur
                        w_cur = ws_t[:, tl, :]
                        if ch == nch - 1 and tl == TCH - 1:
                            _emit_fp32_last_step(
                                nc, const, hpool, pB, pW, sigpool, w_prev,
                                w1xf_d, w1wf_d, w2f_d, w3f_d, b3f_d, xlast_d,
                                b1c, b2c, dw_out,
                            )

                if pending_ws is not None:
                    _pw, _wc, _out = pending_ws
                    nc.vector.scalar_tensor_tensor(
                        out=_out, in0=_pw[:], scalar=b3c[:, 0:1],
                        in1=_wc, op0=ALU.add, op1=ALU.add,
                    )
                    pending_ws = None
                # store W trajectory for this chunk
                nc.sync.dma_start(out=ws_out[ch], in_=ws_t[:])

                # readout: sigmoid(ws0 * x0 + ws1), t-on-partitions layout
                x0_t = sigpool.tile([TCH, BL], f32)
                nc.sync.dma_start(out=x0_t[:], in_=x0[ch])
                wr0 = sigpool.tile([TCH, BL], f32)
                nc.sync.dma_start(out=wr0[:], in_=ws_t[0:1, :, :])
                wr1 = sigpool.tile([TCH, BL], f32)
                nc.sync.dma_start(out=wr1[:], in_=ws_t[1:2, :, :])
                sg = sigpool.tile([TCH, BL], f32)
                nc.vector.tensor_tensor(out=sg[:], in0=wr0[:], in1=x0_t[:], op=ALU.mult)
                nc.vector.tensor_tensor(out=sg[:], in0=sg[:], in1=wr1[:], op=ALU.add)
                st = sigpool.tile([TCH, BL], f32)
                nc.scalar.activation(out=st[:], in_=sg[:], func=AF.Sigmoid)
                nc.sync.dma_start(out=sig_out[ch], in_=st[:])

    if not nc.is_finalized():
        nc.finalize()
    return nc


def _prep_inputs(x, W, scaling_factor, W1, b1, W2, b2, W3, b3, S):
    """Host-side sharding + layout prep. Returns in_maps for the 8 cores."""
    nch = S // TCH
    w1x = np.ascontiguousarray(
        np.concatenate([W1[OUT:], b1.reshape(1, H)], axis=0)
    ).astype(BF)                                                       # (63, 512): [W1x; b1]
    w1w = np.ascontiguousarray(W1[:OUT]).astype(BF)                    # (2, 512)
    b1col = np.ascontiguousarray(b1.reshape(NJ, 128).T).astype(np.float32)
    b2col = np.ascontiguousarray(b2.reshape(NJ, 128).T).astype(np.float32)
    w2 = np.ascontiguousarray(W2.reshape(NJ, 128, H)).astype(BF)
    sf = np.asarray(scaling_factor, np.float32).reshape(1, OUT)
    w3 = np.ascontiguousarray((sf * W3).reshape(NJ, 128, OUT)).astype(BF)
    b3row = np.ascontiguousarray((sf * b3).reshape(1, OUT)).astype(BF)
    b1row = np.ascontiguousarray(b1.reshape(1, H)).astype(BF)
    b2row = np.ascontiguousarray(b2.reshape(1, H)).astype(BF)
    w1x_f = np.ascontiguousarray(W1[OUT:]).astype(np.float32)
    w1w_f = np.ascontiguousarray(W1[:OUT]).astype(np.float32)
    w2_f = np.ascontiguousarray(W2.reshape(NJ, 128, H)).astype(np.float32)
    w3_f = np.ascontiguousarray((sf * W3).reshape(NJ, 128, OUT)).astype(np.float32)
    b3_f = np.ascontiguousarray((sf * b3).reshape(1, OUT)).astype(np.float32)

    in_maps = []
    for c in range(NCORES):
        xc = x[c * BL:(c + 1) * BL, :S]                                # (128, S, 63)
        xm = xc[:, :, 1:].reshape(BL, nch, TCH, F - 1)                 # (b, ch, t, f)
        xm = xm.transpose(1, 3, 2, 0)                                  # (ch, f, t, b)
        xm = np.concatenate(
            [xm, np.ones((nch, 1, TCH, BL), np.float32)], axis=1
        )                                                              # ones plane -> b1
        xm = np.ascontiguousarray(xm).astype(BF)
        x0c = np.ascontiguousarray(
            xc[:, :, 0].reshape(BL, nch, TCH).transpose(1, 2, 0)
        ).astype(np.float32)                                           # (ch, t, b)
        w0tc = np.ascontiguousarray(W[c * BL:(c + 1) * BL, 0, :].T).astype(np.float32)
        x_last = np.ascontiguousarray(xc[:, S - 1, 1:].T).astype(np.float32)
        in_maps.append(dict(
            x_mlp=xm, x0=x0c, w0t=w0tc, w1x=w1x, w1w=w1w, w2=w2, w3=w3,
            b1col=b1col, b2col=b2col, b3row=b3row, b1row=b1row, b2row=b2row,
            b3col=np.ascontiguousarray((sf * b3).reshape(OUT, 1)).astype(np.float32),
            w1x_f=w1x_f, w1w_f=w1w_f, w2_f=w2_f, w3_f=w3_f, b3_f=b3_f,
            x_last=x_last,
        ))
    return in_maps


_NC_CACHE = {}


def run(inputs, S=S_FULL, trace=False, **kw):
    """Run the kernel; returns (sigmoid_output, dW_last, outputs), plus the
    raw BassKernelResults as 4th element."""
    if S not in _NC_CACHE:
        _NC_CACHE[S] = build(S)
    nc = _NC_CACHE[S]
    in_maps = _prep_inputs(S=S, **inputs)
    res = run_bass_kernel_spmd(nc, in_maps, list(range(NCORES)), trace=trace, **kw)

    nch = S // TCH
    outputs = np.empty((B, S, OUT), np.float32)
    sig = np.empty((B, S, 1), np.float32)
    dw_last = np.empty((B, OUT), np.float32)
    for c in range(NCORES):
        ws = res.results[c]["ws_out"]       # (nch, 2, TCH, BL)
        outputs[c * BL:(c + 1) * BL] = ws.transpose(3, 0, 2, 1).reshape(BL, S, OUT)
        sg = res.results[c]["sig_out"]      # (nch, TCH, BL)
        sig[c * BL:(c + 1) * BL, :, 0] = sg.transpose(2, 0, 1).reshape(BL, S)
        dw_last[c * BL:(c + 1) * BL] = res.results[c]["dw_out"].T
    return sig, dw_last, outputs, res


def kernel(**inputs):
    inputs = {k: np.asarray(v) for k, v in inputs.items()}
    sig, dw_last, outputs, _ = run(inputs)
    return sig, dw_last, outputs


# revision 55
# speedup vs baseline: 9.3459x; 9.3459x over previous
"""Trainium2 Bass kernel for nn_DeltaDNNGLM (truncated-BPTT delta-rule GLM forward).

Reference computation (per batch element b, sequential over t = 0..S-1):
    z_t   = concat(W_t, x[b, t, 1:])                # (2 + 62,) = (64,)
    h1    = relu(z_t @ W1 + b1)                     # (512,)
    h2    = relu(h1 @ W2 + b2)                      # (512,)
    dW_t  = 0.001 * (h2 @ W3 + b3)                  # (2,)
    W_t+1 = W_t + dW_t
Outputs:
    outputs[b, t, :]   = W_{t+1}
    sigmoid_output     = sigmoid(outputs[..,0]*x[..,0] + outputs[..,1])
    dW_last            = dW_{S-1}

Strategy: pure data parallel over batch (1024 = 8 cores x 128).  Per core the
activations are kept feature-major: (hidden-on-partitions, batch-on-free), so
every matmul uses the weights as the stationary operand and streams the batch
(N=128).  The x-dependent part of layer 1 is batched 2 timesteps per matmul
(N=256) with b1 folded in via a ones feature-plane; b2/b3 biases ride rank-1
matmuls / the state-update's per-partition scalar.  The W-state recurrence is
kept in SBUF fp32 (plus a bf16 copy feeding layer 1); matmuls run in bf16 with
fp32 PSUM accumulation; dW_last is recomputed fully in fp32 for the final step
(catastrophic cancellation).  Critical chain per step (~3.9 us in the
concourse cost-model timeline; all other work hides under it):
  z-update(DVE) -> L1w(PE,4MM) -> relu1(DVE) -> L2(PE,16MM) -> relu2(DVE)
  -> L3(PE,4MM) -> z-update
"""

import os

import numpy as np
import ml_dtypes

import concourse.bass as bass
import concourse.bacc as bacc
import concourse.mybir as mybir
import concourse.tile as tile
from concourse.bass import ts
from concourse.bass_utils import run_bass_kernel_spmd

B, S_FULL, F, H, OUT = 1024, 1000, 63, 512, 2
NCORES = 8
BL = B // NCORES          # 128 batch per core
TCH = 100                 # chunk (truncation) length
NJ = H // 128             # 4 hidden blocks
TB = 2                    # timesteps batched per layer-1-x matmul

f32 = mybir.dt.float32
bf16 = mybir.dt.bfloat16
BF = ml_dtypes.bfloat16
AF = mybir.ActivationFunctionType
ALU = mybir.AluOpType


def _emit_fp32_last_step(nc, const, hpool, pB, pW, sigpool, w_prev,
                         w1xf_d, w1wf_d, w2f_d, w3f_d, b3f_d, xlast_d,
                         b1c, b2c, dw_out):
    """Recompute dW at t=S-1 entirely in fp32 (only dw_out depends on it)."""
    w1xf = const.tile([F - 1, H], f32)
    nc.sync.dma_start(out=w1xf[:], in_=w1xf_d[:])
    w1wf = const.tile([OUT, H], f32)
    nc.sync.dma_start(out=w1wf[:], in_=w1wf_d[:])
    w2f = const.tile([128, NJ, H], f32)
    nc.sync.dma_start(out=w2f[:], in_=w2f_d[:].rearrange("kb kp j -> kp kb j"))
    w3f = const.tile([128, NJ, OUT], f32)
    nc.sync.dma_start(out=w3f[:], in_=w3f_d[:].rearrange("kb kp o -> kp kb o"))
    b3f = const.tile([1, OUT], f32)
    nc.sync.dma_start(out=b3f[:], in_=b3f_d[:])
    xlast = const.tile([F - 1, BL], f32)
    nc.sync.dma_start(out=xlast[:], in_=xlast_d[:])
    onesf = const.tile([1, BL], f32)
    nc.vector.memset(onesf[:], 1.0)

    paf = pB.tile([128, NJ, BL], f32, tag="pb")
    for j in range(NJ):
        nc.tensor.matmul(paf[:, j], w1xf[:, ts(j, 128)], xlast[:],
                         start=(j == 0), stop=False)
        nc.tensor.matmul(paf[:, j], w1wf[:, ts(j, 128)], w_prev,
                         start=False, stop=True)
    h1f = hpool.tile([128, H], f32, tag="h1f")
    for j in range(NJ):
        nc.scalar.activation(out=h1f[:, ts(j, 128)], in_=paf[:, j],
                             func=AF.Relu, bias=b1c[:, j:j + 1], scale=1.0)
    pbf = pB.tile([128, NJ, BL], f32, tag="pb")
    for kb in range(NJ):
        for j in range(NJ):
            nc.tensor.matmul(pbf[:, j], w2f[:, kb, ts(j, 128)],
                             h1f[:, ts(kb, 128)],
                             start=(kb == 0 and j == 0), stop=(kb == NJ - 1))
    h2f = hpool.tile([128, H], f32, tag="h2f")
    for j in range(NJ):
        nc.scalar.activation(out=h2f[:, ts(j, 128)], in_=pbf[:, j],
                             func=AF.Relu, bias=b2c[:, j:j + 1], scale=1.0)
    pwf = pW.tile([OUT, BL], f32, tag="pw")
    for kb in range(NJ):
        nc.tensor.matmul(pwf[:], w3f[:, kb], h2f[:, ts(kb, 128)],
                         start=(kb == 0), stop=False)
    nc.tensor.matmul(pwf[:], b3f[:], onesf[:], start=False, stop=True)
    dwsb = sigpool.tile([OUT, BL], f32)
    nc.vector.tensor_copy(out=dwsb[:], in_=pwf[:])
    nc.sync.dma_start(out=dw_out[:], in_=dwsb[:])


def build(S=S_FULL, debug=False):
    nch = S // TCH
    nc = bacc.Bacc(None, target_bir_lowering=False)

    # ---- DRAM I/O ----
    # feature rows 0..61 = x[:, t, 1:63]; row 62 = ones (carries b1 via lhsT)
    x_mlp = nc.declare_dram_parameter("x_mlp", [nch, F, TCH, BL], bf16, False)
    x0 = nc.declare_dram_parameter("x0", [nch, TCH, BL], f32, False)
    w0t = nc.declare_dram_parameter("w0t", [OUT, BL], f32, False)
    w1x_d = nc.declare_dram_parameter("w1x", [F, H], bf16, False)  # [W1x; b1]
    w1w_d = nc.declare_dram_parameter("w1w", [OUT, H], bf16, False)
    w2_d = nc.declare_dram_parameter("w2", [NJ, 128, H], bf16, False)
    w3_d = nc.declare_dram_parameter("w3", [NJ, 128, OUT], bf16, False)  # pre-scaled by 1e-3
    b1c_d = nc.declare_dram_parameter("b1col", [128, NJ], f32, False)
    b3c_d = nc.declare_dram_parameter("b3col", [OUT, 1], f32, False)   # pre-scaled
    b2c_d = nc.declare_dram_parameter("b2col", [128, NJ], f32, False)
    b1r_d = nc.declare_dram_parameter("b1row", [1, H], bf16, False)
    b2r_d = nc.declare_dram_parameter("b2row", [1, H], bf16, False)
    b3_d = nc.declare_dram_parameter("b3row", [1, OUT], bf16, False)     # pre-scaled by 1e-3
    # fp32 weight copies + last-step x column for the fp32 shadow of step S-1
    # (dW_last suffers catastrophic cancellation at bf16 precision)
    w1xf_d = nc.declare_dram_parameter("w1x_f", [F - 1, H], f32, False)
    w1wf_d = nc.declare_dram_parameter("w1w_f", [OUT, H], f32, False)
    w2f_d = nc.declare_dram_parameter("w2_f", [NJ, 128, H], f32, False)
    w3f_d = nc.declare_dram_parameter("w3_f", [NJ, 128, OUT], f32, False)
    b3f_d = nc.declare_dram_parameter("b3_f", [1, OUT], f32, False)
    xlast_d = nc.declare_dram_parameter("x_last", [F - 1, BL], f32, False)
    ws_out = nc.declare_dram_parameter("ws_out", [nch, OUT, TCH, BL], f32, True)
    sig_out = nc.declare_dram_parameter("sig_out", [nch, TCH, BL], f32, True)
    dw_out = nc.declare_dram_parameter("dw_out", [OUT, BL], f32, True)
    if debug:
        dbg_pa = nc.declare_dram_parameter("dbg_pa", [128, NJ, BL], f32, True)
        dbg_h1 = nc.declare_dram_parameter("dbg_h1", [128, NJ, BL], f32, True)
        dbg_pb = nc.declare_dram_parameter("dbg_pb", [128, NJ, BL], f32, True)
        dbg_h2 = nc.declare_dram_parameter("dbg_h2", [128, NJ, BL], f32, True)

    with tile.TileContext(nc) as tc:
        with (
            tc.tile_pool(name="const", bufs=1) as const,
            tc.tile_pool(name="xpool", bufs=2) as xpool,
            tc.tile_pool(name="wspool", bufs=2) as wspool,
            tc.tile_pool(name="hpool", bufs=2) as hpool,
            tc.tile_pool(name="sigpool", bufs=2) as sigpool,
            tc.tile_pool(name="pA", bufs=2, space="PSUM") as pA,
            tc.tile_pool(name="pB", bufs=2, space="PSUM") as pB,
            tc.tile_pool(name="pW", bufs=2, space="PSUM") as pW,
        ):
            # ---- load constants ----
            w1x = const.tile([F, H], bf16)
            nc.sync.dma_start(out=w1x[:], in_=w1x_d[:])
            w1w = const.tile([OUT, H], bf16)
            nc.sync.dma_start(out=w1w[:], in_=w1w_d[:])
            b2r = const.tile([1, H], bf16)
            nc.sync.dma_start(out=b2r[:], in_=b2r_d[:])
            w2 = const.tile([128, NJ, H], bf16)
            nc.sync.dma_start(out=w2[:], in_=w2_d[:].rearrange("kb kp j -> kp kb j"))
            w3 = const.tile([128, NJ, OUT], bf16)
            nc.sync.dma_start(out=w3[:], in_=w3_d[:].rearrange("kb kp o -> kp kb o"))
            b1c = const.tile([128, NJ], f32)
            nc.sync.dma_start(out=b1c[:], in_=b1c_d[:])
            b2c = const.tile([128, NJ], f32)
            nc.sync.dma_start(out=b2c[:], in_=b2c_d[:])
            b3r = const.tile([1, OUT], bf16)
            nc.sync.dma_start(out=b3r[:], in_=b3_d[:])
            b3c = const.tile([OUT, 1], f32)
            nc.sync.dma_start(out=b3c[:], in_=b3c_d[:])
            w0sb = const.tile([OUT, BL], f32)
            nc.sync.dma_start(out=w0sb[:], in_=w0t[:])
            ones2 = const.tile([1, TB * BL], bf16)
            nc.vector.memset(ones2[:], 1.0)

            z = const.tile([OUT, BL], bf16)   # bf16 copy of W(t) for layer-1 matmul
            nc.vector.tensor_copy(out=z[:], in_=w0sb[:])
            w_cur = w0sb[:]  # AP of W(t) in SBUF fp32
            pending_ws = None

            for ch in range(nch):
                x_t = xpool.tile([F, TCH, BL], bf16)
                nc.sync.dma_start(out=x_t[:], in_=x_mlp[ch])
                ws_t = wspool.tile([OUT, TCH, BL], f32)

                for tb in range(TCH // TB):
                    pa = pA.tile([128, NJ, TB, BL], f32, tag="pa")
                    # layer-1 x-part: one matmul per hidden block, TB steps
                    # wide. start=True clears the whole PSUM *bank*, so only
                    # the first matmul touching each bank may set it.
                    for j in range(NJ):
                        nc.tensor.matmul(
                            pa[:, j], w1x[:, ts(j, 128)],
                            x_t[:, TB * tb:TB * (tb + 1), :],
                            start=(j % 2 == 0), stop=False,
                        )
                    for tloc in range(TB):
                        tl = TB * tb + tloc
                        # layer-1 W-state part (K=2, bf16 via z state)
                        for j in range(NJ):
                            nc.tensor.matmul(
                                pa[:, j, tloc], w1w[:, ts(j, 128)], z[:],
                                start=False, stop=(tloc == TB - 1),
                            )
                        # b2 bias MMs early: they clear pb's bank and run
                        # in PE's idle window during relu1
                        pb = pB.tile([128, NJ, BL], f32)
                        for j in range(NJ):
                            nc.tensor.matmul(
                                pb[:, j], b2r[:, ts(j, 128)], ones2[:, 0:BL],
                                start=(j == 0), stop=False,
                            )
                        # relu1 -> h1 (bf16); bias already in PSUM. Single ACT op.
                        h1 = hpool.tile([128, NJ, BL], bf16)
                        nc.vector.tensor_scalar(
                            out=h1[:], in0=pa[:, :, tloc, :],
                            scalar1=0.0, scalar2=None, op0=ALU.max,
                        )
                        # layer 2
                        for kb in range(NJ):
                            for j in range(NJ):
                                nc.tensor.matmul(
                                    pb[:, j], w2[:, kb, ts(j, 128)],
                                    h1[:, kb, :],
                                    start=False, stop=(kb == NJ - 1),
                                )
                        if pending_ws is not None:
                            _pw, _wc, _out = pending_ws
                            nc.vector.scalar_tensor_tensor(
                                out=_out, in0=_pw[:], scalar=b3c[:, 0:1],
                                in1=_wc, op0=ALU.add, op1=ALU.add,
                            )
                            pending_ws = None
                        # relu2 -> h2 (bf16), single DVE op
                        h2 = hpool.tile([128, NJ, BL], bf16)
                        nc.vector.tensor_scalar(
                            out=h2[:], in0=pb[:],
                            scalar1=0.0, scalar2=None, op0=ALU.max,
                        )
                        if debug and ch == 0 and tl == 0:
                            dpa = sigpool.tile([128, NJ, BL], f32, tag="dbgpa")
                            nc.vector.tensor_copy(out=dpa[:], in_=pa[:, :, tloc, :])
                            nc.sync.dma_start(out=dbg_pa[:], in_=dpa[:])
                            dh1 = sigpool.tile([128, NJ, BL], f32, tag="dbgh1")
                            nc.vector.tensor_copy(out=dh1[:], in_=h1[:])
                            nc.sync.dma_start(out=dbg_h1[:], in_=dh1[:])
                            dpb = sigpool.tile([128, NJ, BL], f32, tag="dbgpb")
                            nc.vector.tensor_copy(out=dpb[:], in_=pb[:])
                            nc.sync.dma_start(out=dbg_pb[:], in_=dpb[:])
                            dh2 = sigpool.tile([128, NJ, BL], f32, tag="dbgh2")
                            nc.vector.tensor_copy(out=dh2[:], in_=h2[:])
                            nc.sync.dma_start(out=dbg_h2[:], in_=dh2[:])
                        # layer 3 (+ scaled bias via ones row) -> dW in PSUM
                        pw = pW.tile([OUT, BL], f32)
                        for kb in range(NJ):
                            nc.tensor.matmul(
                                pw[:], w3[:, kb], h2[:, kb, :],
                                start=(kb == 0), stop=(kb == NJ - 1),
                            )
                        # state update: z (bf16, critical path) then ws (fp32)
                        nc.vector.scalar_tensor_tensor(
                            out=z[:], in0=pw[:], scalar=b3c[:, 0:1],
                            in1=w_cur, op0=ALU.add, op1=ALU.add,
                        )
                        pending_ws = (pw, w_cur, ws_t[:, tl, :])
                        w_prev = w_cur
                        w_cur = ws_t[:, tl, :]
                        if ch == nch - 1 and tl == TCH - 1:
                            _emit_fp32_last_step(
                                nc, const, hpool, pB, pW, sigpool, w_prev,
                                w1xf_d, w1wf_d, w2f_d, w3f_d, b3f_d, xlast_d,
                                b1c, b2c, dw_out,
                            )

                if pending_ws is not None:
                    _pw, _wc, _out = pending_ws
                    nc.vector.scalar_tensor_tensor(
                        out=_out, in0=_pw[:], scalar=b3c[:, 0:1],
                        in1=_wc, op0=ALU.add, op1=ALU.add,
                    )
                    pending_ws = None
                # store W trajectory for this chunk
                nc.sync.dma_start(out=ws_out[ch], in_=ws_t[:])

                # readout: sigmoid(ws0 * x0 + ws1), t-on-partitions layout
                x0_t = sigpool.tile([TCH, BL], f32)
                nc.sync.dma_start(out=x0_t[:], in_=x0[ch])
                wr0 = sigpool.tile([TCH, BL], f32)
                nc.sync.dma_start(out=wr0[:], in_=ws_t[0:1, :, :])
                wr1 = sigpool.tile([TCH, BL], f32)
                nc.sync.dma_start(out=wr1[:], in_=ws_t[1:2, :, :])
                sg = sigpool.tile([TCH, BL], f32)
                nc.vector.tensor_tensor(out=sg[:], in0=wr0[:], in1=x0_t[:], op=ALU.mult)
                nc.vector.tensor_tensor(out=sg[:], in0=sg[:], in1=wr1[:], op=ALU.add)
                st = sigpool.tile([TCH, BL], f32)
                nc.scalar.activation(out=st[:], in_=sg[:], func=AF.Sigmoid)
                nc.sync.dma_start(out=sig_out[ch], in_=st[:])

    if not nc.is_finalized():
        nc.finalize()
    return nc


def _prep_inputs(x, W, scaling_factor, W1, b1, W2, b2, W3, b3, S):
    """Host-side sharding + layout prep. Returns in_maps for the 8 cores."""
    nch = S // TCH
    w1x = np.ascontiguousarray(
        np.concatenate([W1[OUT:], b1.reshape(1, H)], axis=0)
    ).astype(BF)                                                       # (63, 512): [W1x; b1]
    w1w = np.ascontiguousarray(W1[:OUT]).astype(BF)                    # (2, 512)
    b1col = np.ascontiguousarray(b1.reshape(NJ, 128).T).astype(np.float32)
    b2col = np.ascontiguousarray(b2.reshape(NJ, 128).T).astype(np.float32)
    w2 = np.ascontiguousarray(W2.reshape(NJ, 128, H)).astype(BF)
    sf = np.asarray(scaling_factor, np.float32).reshape(1, OUT)
    w3 = np.ascontiguousarray((sf * W3).reshape(NJ, 128, OUT)).astype(BF)
    b3row = np.ascontiguousarray((sf * b3).reshape(1, OUT)).astype(BF)
    b1row = np.ascontiguousarray(b1.reshape(1, H)).astype(BF)
    b2row = np.ascontiguousarray(b2.reshape(1, H)).astype(BF)
    w1x_f = np.ascontiguousarray(W1[OUT:]).astype(np.float32)
    w1w_f = np.ascontiguousarray(W1[:OUT]).astype(np.float32)
    w2_f = np.ascontiguousarray(W2.reshape(NJ, 128, H)).astype(np.float32)
    w3_f = np.ascontiguousarray((sf * W3).reshape(NJ, 128, OUT)).astype(np.float32)
    b3_f = np.ascontiguousarray((sf * b3).reshape(1, OUT)).astype(np.float32)

    in_maps = []
    for c in range(NCORES):
        xc = x[c * BL:(c + 1) * BL, :S]                                # (128, S, 63)
        xm = xc[:, :, 1:].reshape(BL, nch, TCH, F - 1)                 # (b, ch, t, f)
        xm = xm.transpose(1, 3, 2, 0)                                  # (ch, f, t, b)
        xm = np.concatenate(
            [xm, np.ones((nch, 1, TCH, BL), np.float32)], axis=1
        )                                                              # ones plane -> b1
        xm = np.ascontiguousarray(xm).astype(BF)
        x0c = np.ascontiguousarray(
            xc[:, :, 0].reshape(BL, nch, TCH).transpose(1, 2, 0)
        ).astype(np.float32)                                           # (ch, t, b)
        w0tc = np.ascontiguousarray(W[c * BL:(c + 1) * BL, 0, :].T).astype(np.float32)
        x_last = np.ascontiguousarray(xc[:, S - 1, 1:].T).astype(np.float32)
        in_maps.append(dict(
            x_mlp=xm, x0=x0c, w0t=w0tc, w1x=w1x, w1w=w1w, w2=w2, w3=w3,
            b1col=b1col, b2col=b2col, b3row=b3row, b1row=b1row, b2row=b2row,
            b3col=np.ascontiguousarray((sf * b3).reshape(OUT, 1)).astype(np.float32),
            w1x_f=w1x_f, w1w_f=w1w_f, w2_f=w2_f, w3_f=w3_f, b3_f=b3_f,
            x_last=x_last,
        ))
    return in_maps


_NC_CACHE = {}


def run(inputs, S=S_FULL, trace=False, **kw):
    """Run the kernel; returns (sigmoid_output, dW_last, outputs), plus the
    raw BassKernelResults as 4th element."""
    if S not in _NC_CACHE:
        _NC_CACHE[S] = build(S)
    nc = _NC_CACHE[S]
    in_maps = _prep_inputs(S=S, **inputs)
    res = run_bass_kernel_spmd(nc, in_maps, list(range(NCORES)), trace=trace, **kw)

    nch = S // TCH
    outputs = np.empty((B, S, OUT), np.float32)
    sig = np.empty((B, S, 1), np.float32)
    dw_last = np.empty((B, OUT), np.float32)
    for c in range(NCORES):
        ws = res.results[c]["ws_out"]       # (nch, 2, TCH, BL)
        outputs[c * BL:(c + 1) * BL] = ws.transpose(3, 0, 2, 1).reshape(BL, S, OUT)
        sg = res.results[c]["sig_out"]      # (nch, TCH, BL)
        sig[c * BL:(c + 1) * BL, :, 0] = sg.transpose(2, 0, 1).reshape(BL, S)
        dw_last[c * BL:(c + 1) * BL] = res.results[c]["dw_out"].T
    return sig, dw_last, outputs, res


def kernel(**inputs):
    inputs = {k: np.asarray(v) for k, v in inputs.items()}
    sig, dw_last, outputs, _ = run(inputs)
    return sig, dw_last, outputs
